# revision 24
# baseline (speedup 1.0000x reference)
"""Trainium2 Bass kernel for a dense transformer encoder block.

Shards across 8 NeuronCores with no collectives: core c handles batch
b=c//2 and query-half qh=c%2 (1024 query rows). K/V are recomputed per
core over the full 2048-row sequence of its batch.

Structure (v2):
- Host pre-transposes x (xt input), folds bv@Wo+bo into the residual
  input xq, g1 into W1, be1 into b1/b2. All exact algebra.
- Phase A: QKV projections split into quanta interleaved between
  attention chunks so the PE never lumps projection work while the
  scalar engine (exp pacemaker) starves.
- Phase B: Wo + residual + LN1 with square/normalize on the scalar
  engine, h kept bf16, bf16 PE transposes for the FFN layout.
- Phase C: FFN1 (u fully resident), FFN2 tq-outer with 32-matmul PSUM
  chains; LN2 + store pipelined under the FFN2 matmuls. W2 prefetched
  on the scalar engine's DMA queue into buffers freed by earlier
  phases.

Numerics: bf16 storage/matmul operands with fp32 PSUM accumulation and
fp32 softmax/layernorm statistics.

Self-contained: needs numpy + the concourse tree at /opt/trn_rl_repo.
"""

import sys

if "/opt/trn_rl_repo" not in sys.path:
    sys.path.insert(0, "/opt/trn_rl_repo")

import numpy as np

B, S, D, H, DK, FFN = 4, 2048, 1024, 16, 64, 4096
P = 128            # partitions
NSQ = S // 2       # local query rows per core (1024)
HP = H // 2        # head pairs (8)
DC = D // P        # d_model chunks (8)
SC = S // P        # sequence chunks (16)
FC = FFN // P      # ffn chunks (32)
EPS = 1e-5

# gbe vector order in the stacked [4, 1, D] input
VG1, VB2, VG2, VBE2 = range(4)

_compiled = None


def _build():
    import os
    PH = int(os.environ.get("KERNEL_PHASES", "9"))
    import concourse.bacc as bacc
    import concourse.tile as tile
    import concourse.mybir as mybir
    from concourse.masks import make_identity

    f32 = mybir.dt.float32
    f32r = mybir.dt.float32r
    bf16 = mybir.dt.bfloat16
    ACT = mybir.ActivationFunctionType
    ALU = mybir.AluOpType

    nc = bacc.Bacc("TRN2", target_bir_lowering=False, debug=False,
                   enable_asserts=False, num_devices=8)

    xt_d = nc.dram_tensor("xt", [P, 4, DC, 512], bf16, kind="ExternalInput")
    xq_d = nc.dram_tensor("xq", [P, DC, D], bf16, kind="ExternalInput")
    wq_d = nc.dram_tensor("wq", [HP, P, DC, P], bf16, kind="ExternalInput")
    wk_d = nc.dram_tensor("wk", [HP, P, DC, P], bf16, kind="ExternalInput")
    wv_d = nc.dram_tensor("wv", [4, P, DC, 256], bf16, kind="ExternalInput")
    bq_d = nc.dram_tensor("bq", [P, HP], f32, kind="ExternalInput")
    bk_d = nc.dram_tensor("bk", [P, HP], f32, kind="ExternalInput")
    wo_d = nc.dram_tensor("wo", [P, DC, D], bf16, kind="ExternalInput")
    w1_d = nc.dram_tensor("w1", [FC, P, DC, P], bf16, kind="ExternalInput")
    b1_d = nc.dram_tensor("b1", [P, FC], f32, kind="ExternalInput")
    w2_d = nc.dram_tensor("w2", [4, P, DC, D], bf16, kind="ExternalInput")
    gbe_d = nc.dram_tensor("gbe", [4, 1, D], bf16, kind="ExternalInput")
    out_d = nc.dram_tensor("out", [P, DC, D], f32, kind="ExternalOutput")

    UDIM = [P, DC, NSQ]   # 16KB arena unit shape (bf16)

    with tile.TileContext(nc) as tc:
        with (
            tc.tile_pool(name="const", bufs=1) as const,
            tc.tile_pool(name="vbc", bufs=1) as vbc,
            tc.tile_pool(name="arena", bufs=1) as arena,
            tc.tile_pool(name="v4p", bufs=1) as v4p,
            tc.tile_pool(name="w1p", bufs=3) as w1p,
            tc.tile_pool(name="small1", bufs=1) as small1,
            tc.tile_pool(name="work", bufs=2) as work,
            tc.tile_pool(name="ps_acc", bufs=2, space="PSUM") as ps_acc,
            tc.tile_pool(name="ps_sc", bufs=2, space="PSUM") as ps_sc,
            tc.tile_pool(name="ps_zt", bufs=2, space="PSUM") as ps_zt,
        ):
            ident_bf = const.tile([P, P], bf16)
            make_identity(nc, ident_bf[:])
            eps_sb = const.tile([P, 1], f32)
            nc.gpsimd.memset(eps_sb[:], float(EPS))
            ones64f = const.tile([1, DK], f32)
            nc.gpsimd.memset(ones64f[:], 1.0)
            ones64 = const.tile([1, DK], f32r)
            nc.vector.tensor_copy(ones64[:], ones64f[:])
            bq_sb = const.tile([P, HP], f32)
            bk_sb = const.tile([P, HP], f32)
            b1_sb = const.tile([P, FC], f32)
            nc.sync.dma_start(bq_sb[:], bq_d[:])
            nc.sync.dma_start(bk_sb[:], bk_d[:])
            nc.sync.dma_start(b1_sb[:], b1_d[:])

            def make_bcast(idx, tag):
                # broadcast gbe_d[idx] ([1, D]) to [P, D]
                t = vbc.tile([P, D], bf16, tag=tag)
                for sl in range(2):
                    vs = small1.tile([1, 512], bf16, tag="l1", name="vs")
                    nc.sync.dma_start(vs[:],
                                      gbe_d[idx][:, sl * 512:(sl + 1) * 512])
                    nc.gpsimd.partition_broadcast(
                        t[:, sl * 512:(sl + 1) * 512], vs[:])
                return t

            # ---- arena unit tiles (16KB each); tags chain across phases
            xt_lo = arena.tile(UDIM, bf16, tag="A1")   # x^T cols 0..1023
            xt_hi = arena.tile(UDIM, bf16, tag="A2")   # x^T cols 1024..2047
            wk_sb = arena.tile([P, HP, DC, P], bf16, tag="A3")
            wq_sb = arena.tile([P, HP, DC, P], bf16, tag="A4")
            wv_sb = arena.tile([P, 4, DC, 256], bf16, tag="A5")
            ct = arena.tile([P, HP, NSQ], bf16, tag="A6")  # concat^T
            ktqa = arena.tile(UDIM, bf16, tag="A7")  # kt x2, qt x2, at x2

            # x^T DMA split by 512-col chunks so the first projection
            # quantum unblocks early
            for st in range(2):
                nc.sync.dma_start(xt_lo[:, :, st * 512:(st + 1) * 512],
                                  xt_d[:, st])
            nc.sync.dma_start(wk_sb[:, 0], wk_d[0])
            nc.sync.dma_start(wq_sb[:, 0], wq_d[0])
            for st in range(2, 4):
                nc.sync.dma_start(
                    xt_hi[:, :, (st - 2) * 512:(st - 1) * 512],
                    xt_d[:, st])
            nc.sync.dma_start(wv_sb[:, 0], wv_d[0])
            for hp in range(1, HP):
                nc.sync.dma_start(wk_sb[:, hp], wk_d[hp])
                nc.sync.dma_start(wq_sb[:, hp], wq_d[hp])
            for g in range(1, 4):
                nc.sync.dma_start(wv_sb[:, g], wv_d[g])

            # views into the ktqa unit
            def kt_view(hp):
                j = (hp % 2) * 2
                return ktqa[:, j:j + 2, :].rearrange("p a b -> p (a b)")

            def qt_view(hp):
                return ktqa[:, 4 + hp % 2, :]

            def at_view(c):
                return ktqa[:, 6 + c % 2, :].rearrange(
                    "p (h q) -> p h q", h=2)

            def xts(st):
                # [P, DC, 512] view of x^T for sequence 512-chunk st
                src = xt_lo if st < 2 else xt_hi
                return src[:, :, (st % 2) * 512:(st % 2 + 1) * 512]

            def xtc(sc):
                # [P, DC, 128] view of x^T for sequence 128-chunk sc
                src = xt_lo if sc < 8 else xt_hi
                j = sc % 8
                return src[:, :, j * P:(j + 1) * P]

            v4g = [None, None]

            def v4_tile(g):
                t = v4p.tile([P, SC, 4, DK + 1], bf16, tag=f"V{g % 2}")
                nc.vector.memset(t[:, :, :, DK], 1.0)
                v4g[g % 2] = t
                return t

            # ---- projection quanta
            def k_quantum(hp, st):
                def go():
                    pk = ps_acc.tile([P, 512], f32, tag="acc")
                    for dc in range(DC):
                        nc.tensor.matmul(pk[:],
                                         wk_sb[:, hp, dc, :],
                                         xts(st)[:, dc, :],
                                         start=(dc == 0), stop=(dc == DC - 1))
                    nc.vector.tensor_scalar(
                        out=kt_view(hp)[:, st * 512:(st + 1) * 512],
                        in0=pk[:], scalar1=bk_sb[:, hp:hp + 1], scalar2=None,
                        op0=ALU.add)
                return go

            def q_quantum(hp, st):
                def go():
                    pq = ps_acc.tile([P, 512], f32, tag="acc")
                    for dc in range(DC):
                        nc.tensor.matmul(pq[:],
                                         wq_sb[:, hp, dc, :],
                                         xts(st)[:, dc, :],
                                         start=(dc == 0), stop=(dc == DC - 1))
                    nc.vector.tensor_scalar(
                        out=qt_view(hp)[:, st * 512:(st + 1) * 512],
                        in0=pq[:], scalar1=bq_sb[:, hp:hp + 1], scalar2=None,
                        op0=ALU.add)
                return go

            def v_quantum(g, sc):
                def go():
                    pv = ps_acc.tile([P, 256], f32, tag="acc")
                    for dc in range(DC):
                        nc.tensor.matmul(pv[:], xtc(sc)[:, dc, :],
                                         wv_sb[:, g, dc, :],
                                         start=(dc == 0), stop=(dc == DC - 1))
                    nc.vector.tensor_copy(
                        v4g[g % 2][:, sc, :, 0:DK],
                        pv[:].rearrange("p (h k) -> p h k", h=4))
                return go

            pending = []

            def pump(n=1):
                for _ in range(n):
                    if pending:
                        pending.pop(0)()

            # prologue projections for hp=0 (+ first V chunks of group 0)
            if PH >= 2:
                for st in range(4):
                    k_quantum(0, st)()
                for st in range(2):
                    q_quantum(0, st)()
                v4_tile(0)
                for sc in range(4):
                    v_quantum(0, sc)()
                pending += [v_quantum(0, sc) for sc in range(4, SC)]

            # ---- phase A: attention, software-pipelined one chunk deep:
            # scores(i+1) is emitted before Z(i) so the PE computes the next
            # chunk's scores while the scalar engine runs exp(i).
            def normalize_ct(hp, l4):
                # divide hp's unnormalized Z^T slices by their softmax sums;
                # l rows live at 32-aligned partitions of l4. The [DK, 512]
                # partition-broadcast of each reciprocal row runs as a K=1
                # ones-column matmul on the PE (no gpsimd round-trip).
                nc.vector.reciprocal(l4[:], l4[:])
                for j in range(4):
                    sq_t, h = divmod(j, 2)
                    l1 = small1.tile([1, 512], f32r, tag="l1", name="l1")
                    nc.vector.tensor_copy(l1[:], l4[32 * j:32 * j + 1, :])
                    bc_l = ps_acc.tile([DK, 512], f32, tag="acc", name="bc")
                    nc.tensor.matmul(bc_l[:], ones64[:], l1[:],
                                     start=True, stop=True)
                    csl = ct[h * DK:(h + 1) * DK, hp,
                             sq_t * 512:(sq_t + 1) * 512]
                    nc.vector.tensor_tensor(csl, csl, bc_l[:], ALU.mult)

            chunks = [(hp, sq_t, c) for hp in range(HP)
                      for sq_t in range(NSQ // 512) for c in range(SC)]
            if PH < 2:
                chunks = []

            def emit_scores(hp, sq_t, c):
                scp = ps_sc.tile([P, 2, 512], f32, tag="sc", name="scp")
                kt_hp = kt_view(hp)
                qt_hp = qt_view(hp)
                for h in range(2):
                    nc.tensor.matmul(
                        scp[:, h, :],
                        kt_hp[h * DK:(h + 1) * DK, c * P:(c + 1) * P],
                        qt_hp[h * DK:(h + 1) * DK,
                              sq_t * 512:(sq_t + 1) * 512],
                        start=True, stop=True)
                return scp

            zts = None
            l4_cur = None
            l4_prev = None
            scp_cur = emit_scores(*chunks[0]) if chunks else None
            for i, (hp, sq_t, c) in enumerate(chunks):
                if c == 0:
                    if sq_t == 0:
                        if hp > 0:
                            normalize_ct(hp - 1, l4_prev)
                        l4_cur = small1.tile(
                            [97, 512], f32, tag="l4", name="l4")
                        nc.vector.memset(l4_cur[:], 1.0)
                        l4_prev = l4_cur
                    if hp < HP - 1 and sq_t == 0:
                        pending.extend(k_quantum(hp + 1, st)
                                       for st in range(4))
                        pending.extend(q_quantum(hp + 1, st)
                                       for st in range(2))
                        if hp % 2 == 1:
                            v4_tile(hp // 2 + 1)
                            pending.extend(v_quantum(hp // 2 + 1, sc)
                                           for sc in range(SC))
                    zt0 = ps_zt.tile([DK + 1, 512], f32, tag="zt")
                    zt1 = ps_zt.tile([DK + 1, 512], f32, tag="zt")
                    zts = (zt0, zt1)
                at = at_view(c)
                nc.scalar.activation(at, scp_cur[:], ACT.Exp, scale=0.125)
                if i + 1 < len(chunks):
                    scp_cur = emit_scores(*chunks[i + 1])
                for h in range(2):
                    nc.tensor.matmul(
                        zts[h][:],
                        v4g[(hp // 2) % 2][:, c, (hp % 2) * 2 + h, :],
                        at[:, h, :],
                        start=(c == 0), stop=(c == SC - 1))
                if c == SC - 1:
                    for h in range(2):
                        j = sq_t * 2 + h
                        nc.vector.tensor_copy(
                            ct[h * DK:(h + 1) * DK, hp,
                               sq_t * 512:(sq_t + 1) * 512],
                            zts[h][0:DK, :])
                        nc.vector.tensor_copy(
                            l4_cur[32 * j:32 * j + 1, :],
                            zts[h][DK:DK + 1, :])
                pump(1)

            if PH >= 2:
                pump(len(pending))
                normalize_ct(HP - 1, l4_prev)

            # late-phase weight/data loads into freed arena units
            wo_sb = arena.tile([P, DC, D], bf16, tag="A3")   # over wk
            xq_sb = arena.tile([P, DC, NSQ], bf16, tag="A4")  # over wq
            h_core = arena.tile([P, DC, D], bf16, tag="A5")   # over wv
            if PH >= 3:
                nc.sync.dma_start(wo_sb[:], wo_d[:])
                nc.sync.dma_start(xq_sb[:], xq_d[:])

            # W2 units: prefetched via the scalar engine's DMA queue into
            # buffers freed as phases retire (A1/A2 after projections,
            # A3/A4 after phase B).
            w2u = [arena.tile([P, DC, D], bf16, tag=t, name=f"w2u{t}")
                   for t in ("A1", "A2", "A3", "A4")]
            if PH >= 5:
                for j in range(2):
                    nc.scalar.dma_start(w2u[j][:], w2_d[j])

            # uts units (u^T, relu(W1^T h^T)): A6 freed after Wo, plus
            # three fresh units
            uts = [arena.tile([P, DC, NSQ], bf16, tag=t, name=f"uts{t}")
                   for t in ("A6", "A8", "A9", "A10")]
            ht = arena.tile(UDIM, bf16, tag="A7")  # over ktqa

            g1b = make_bcast(VG1, "g1b") if PH >= 5 else None
            b2b = make_bcast(VB2, "b2b") if PH >= 5 else None
            g2b = make_bcast(VG2, "g2b") if PH >= 5 else None
            be2b = make_bcast(VBE2, "be2b") if PH >= 5 else None

            def ln_stats(res_parts):
                """res_parts: [(res [P,512] f32, rowsum [P,1]), ...2] ->
                (rs [P,1], nmu [P,1]) via scalar-engine square/sqrt."""
                (r0, s0), (r1, s1) = res_parts
                mu = work.tile([P, 1], f32, tag="mu")
                nc.vector.tensor_tensor(mu[:], s0[:], s1[:], ALU.add)
                nc.vector.tensor_scalar_mul(mu[:], mu[:], 1.0 / D)
                ssq0 = work.tile([P, 1], f32, tag="ssq0")
                ssq1 = work.tile([P, 1], f32, tag="ssq1")
                for rsl, ssq in ((r0, ssq0), (r1, ssq1)):
                    sqz = ps_zt.tile([P, 512], f32, tag="zt")
                    nc.scalar.activation(sqz[:], rsl[:], ACT.Square,
                                         accum_out=ssq[:])
                var = work.tile([P, 1], f32, tag="var")
                nc.vector.tensor_tensor(var[:], ssq0[:], ssq1[:], ALU.add)
                nc.vector.tensor_scalar_mul(var[:], var[:], 1.0 / D)
                musq = work.tile([P, 1], f32, tag="musq")
                nc.vector.tensor_mul(musq[:], mu[:], mu[:])
                nc.vector.tensor_sub(var[:], var[:], musq[:])
                sd = work.tile([P, 1], f32, tag="sd")
                nc.scalar.activation(sd[:], var[:], ACT.Sqrt, bias=eps_sb[:])
                rs = work.tile([P, 1], f32, tag="rs")
                nc.vector.reciprocal(rs[:], sd[:])
                nmu = work.tile([P, 1], f32, tag="nmu")
                nc.vector.tensor_mul(nmu[:], mu[:], rs[:])
                nc.vector.tensor_scalar_mul(nmu[:], nmu[:], -1.0)
                return rs, nmu

            # ---- phase B: Wo + residual + LN1 -> h_core (bf16), h^T
            def transpose_tq(tq):
                for dq in range(2):
                    tp = ps_sc.tile([P, 4, P], bf16, tag="sc")
                    for j in range(4):
                        dc = dq * 4 + j
                        nc.tensor.transpose(
                            tp[:, j, :],
                            h_core[:, tq, dc * P:(dc + 1) * P],
                            ident_bf[:])
                    nc.vector.tensor_copy(
                        ht[:, dq * 4:(dq + 1) * 4, tq * P:(tq + 1) * P],
                        tp[:])

            for tq in range(DC if PH >= 3 else 0):
                res_parts = []
                for sl in range(2):
                    pa = ps_acc.tile([P, 512], f32, tag="acc")
                    for hp in range(HP):
                        nc.tensor.matmul(
                            pa[:], ct[:, hp, tq * P:(tq + 1) * P],
                            wo_sb[:, hp, sl * 512:(sl + 1) * 512],
                            start=(hp == 0), stop=(hp == HP - 1))
                    res = work.tile([P, 512], f32, tag=f"r{sl}")
                    rsum = work.tile([P, 1], f32, tag=f"rsum{sl}")
                    nc.vector.scalar_tensor_tensor(
                        out=res[:], in0=pa[:], scalar=1.0,
                        in1=xq_sb[:, tq, sl * 512:(sl + 1) * 512],
                        op0=ALU.mult, op1=ALU.add, accum_out=rsum[:])
                    res_parts.append((res, rsum))
                rs, nmu = ln_stats(res_parts)
                for sl in range(2):
                    nc.scalar.activation(
                        h_core[:, tq, sl * 512:(sl + 1) * 512],
                        res_parts[sl][0][:], ACT.Identity,
                        bias=nmu[:], scale=rs[:])
                if PH >= 4 and tq > 0:
                    transpose_tq(tq - 1)
            if PH >= 4:
                transpose_tq(DC - 1)

            # ---- phase C: FFN1 (full u resident), then tq-outer FFN2+LN2
            for ft in range(FC if PH >= 4 else 0):
                w1t = w1p.tile([P, DC, P], bf16, tag="w1")
                nc.scalar.dma_start(w1t[:], w1_d[ft])
                for st in range(2):
                    pu = ps_acc.tile([P, 512], f32, tag="acc")
                    for dc in range(DC):
                        nc.tensor.matmul(
                            pu[:], w1t[:, dc, :],
                            ht[:, dc, st * 512:(st + 1) * 512],
                            start=(dc == 0), stop=(dc == DC - 1))
                    nc.vector.tensor_scalar(
                        out=uts[ft // DC][:, ft % DC,
                                          st * 512:(st + 1) * 512],
                        in0=pu[:], scalar1=b1_sb[:, ft:ft + 1],
                        scalar2=0.0, op0=ALU.add, op1=ALU.max)
                if PH >= 5 and ft == 0:
                    for j in range(2, 4):
                        nc.scalar.dma_start(w2u[j][:], w2_d[j])

            for tq in range(DC if PH >= 5 else 0):
                res_parts = []
                for sl in range(2):
                    py = ps_acc.tile([P, 512], f32, tag="acc")
                    for fc in range(FC):
                        nc.tensor.matmul(
                            py[:],
                            uts[fc // DC][:, fc % DC, tq * P:(tq + 1) * P],
                            w2u[fc // DC][:, fc % DC,
                                          sl * 512:(sl + 1) * 512],
                            start=(fc == 0), stop=(fc == FC - 1))
                    hs = h_core[:, tq, sl * 512:(sl + 1) * 512]
                    t0 = work.tile([P, 512], f32, tag="t0", bufs=1)
                    nc.vector.tensor_tensor(
                        t0[:], hs, g1b[:, sl * 512:(sl + 1) * 512], ALU.mult)
                    nc.vector.tensor_tensor(
                        t0[:], t0[:], b2b[:, sl * 512:(sl + 1) * 512],
                        ALU.add)
                    z = work.tile([P, 512], f32, tag=f"r{sl}")
                    rsum = work.tile([P, 1], f32, tag=f"rsum{sl}")
                    nc.vector.scalar_tensor_tensor(
                        out=z[:], in0=py[:], scalar=1.0, in1=t0[:],
                        op0=ALU.mult, op1=ALU.add, accum_out=rsum[:])
                    res_parts.append((z, rsum))
                rs, nmu = ln_stats(res_parts)
                for sl in range(2):
                    z = res_parts[sl][0]
                    nc.scalar.activation(z[:], z[:], ACT.Identity,
                                         bias=nmu[:], scale=rs[:])
                    nc.vector.tensor_tensor(
                        z[:], z[:], g2b[:, sl * 512:(sl + 1) * 512],
                        ALU.mult)
                    nc.vector.tensor_tensor(
                        z[:], z[:], be2b[:, sl * 512:(sl + 1) * 512],
                        ALU.add)
                    nc.sync.dma_start(
                        out_d[:, tq, sl * 512:(sl + 1) * 512], z[:])

    nc.compile()
    return nc


def _get_compiled():
    global _compiled
    if _compiled is None:
        _compiled = _build()
    return _compiled


def _host_inputs(inputs):
    """Shared (per-core-identical) weight arrays in kernel layout.

    All big tensors are pre-arranged into the exact SBUF layouts so every
    device DMA is a flat per-partition contiguous copy."""
    import ml_dtypes
    f = np.float32
    bf = ml_dtypes.bfloat16
    cat = lambda w: np.ascontiguousarray(
        np.transpose(np.asarray(w, f), (1, 0, 2)).reshape(D, D))
    vec = lambda k: np.asarray(inputs[k], f).reshape(D)
    Wo = np.asarray(inputs["Wo"], f)
    W1 = np.asarray(inputs["W1"], f)
    # folds: bv@Wo+bo -> residual input (see make_in_maps), g1 -> W1,
    # be1 -> b1/b2
    g1 = vec("g1")
    be1 = vec("be1")
    W1f = W1 * g1[:, None]
    b1f = np.asarray(inputs["b1"], f).reshape(FFN) + be1 @ W1
    b2f = vec("b2") + be1
    gbe = np.stack([vec("g1").reshape(1, D), b2f.reshape(1, D),
                    vec("g2").reshape(1, D), vec("be2").reshape(1, D)],
                   axis=0).astype(bf)
    wq_cat = cat(inputs["Wq"])
    wk_cat = cat(inputs["Wk"])
    wv_cat = cat(inputs["Wv"])
    # [HP, P, DC, P]: w[hp, p, n, m] = cat[n*128+p, hp*128+m]
    whp = lambda w: np.ascontiguousarray(
        w.reshape(DC, P, HP, P).transpose(2, 1, 0, 3).astype(bf))
    # [4, P, DC, 256]
    wvg = np.ascontiguousarray(
        wv_cat.reshape(DC, P, 4, 256).transpose(2, 1, 0, 3).astype(bf))
    return {
        "wq": whp(wq_cat),
        "wk": whp(wk_cat),
        "wv": wvg,
        "bq": np.ascontiguousarray(
            np.asarray(inputs["bq"], f).reshape(HP, P).T),
        "bk": np.ascontiguousarray(
            np.asarray(inputs["bk"], f).reshape(HP, P).T),
        "wo": np.ascontiguousarray(
            Wo.reshape(DC, P, D).transpose(1, 0, 2).astype(bf)),
        "w1": np.ascontiguousarray(
            W1f.reshape(DC, P, FC, P).transpose(2, 1, 0, 3).astype(bf)),
        "b1": np.ascontiguousarray(np.asarray(b1f, f).reshape(FC, P).T),
        "w2": np.ascontiguousarray(
            np.asarray(inputs["W2"], f).reshape(4, DC, P, D)
            .transpose(0, 2, 1, 3).astype(bf)),
        "gbe": np.ascontiguousarray(gbe),
    }


def make_in_maps(inputs):
    import ml_dtypes
    bf = ml_dtypes.bfloat16
    shared = _host_inputs(inputs)
    x = np.asarray(inputs["x"], np.float32)
    bo_eff = (np.asarray(inputs["bo"], np.float32)
              + np.asarray(inputs["bv"], np.float32).reshape(D)
              @ np.asarray(inputs["Wo"], np.float32))
    in_maps = []
    for c in range(8):
        b, qh = c // 2, c % 2
        if qh == 0:
            xb = x[b]
        else:
            xb = np.concatenate([x[b, NSQ:], x[b, :NSQ]], axis=0)
        xT = xb.T  # [D, S]
        xq = xb[:NSQ] + bo_eff  # [NSQ, D]
        in_maps.append({
            "xt": np.ascontiguousarray(
                xT.reshape(DC, P, 4, 512).transpose(1, 2, 0, 3).astype(bf)),
            "xq": np.ascontiguousarray(
                xq.reshape(DC, P, D).transpose(1, 0, 2).astype(bf)),
            **shared,
        })
    return in_maps


def assemble(results):
    out = np.empty((B, S, D), np.float32)
    for c in range(8):
        b, qh = c // 2, c % 2
        o = np.asarray(results[c]["out"])  # [P, DC, D]
        out[b, qh * NSQ:(qh + 1) * NSQ, :] = (
            o.transpose(1, 0, 2).reshape(NSQ, D))
    return out


def run_on_hw(inputs, trace=False, tmpdir=None):
    from concourse.bass_utils import run_bass_kernel_spmd
    nc = _get_compiled()
    res = run_bass_kernel_spmd(nc, make_in_maps(inputs), list(range(8)),
                               trace=trace, tmpdir=tmpdir)
    return assemble(res.results), res


def kernel(**inputs):
    out, _ = run_on_hw(inputs)
    return out


# revision 27
# speedup vs baseline: 1.0894x; 1.0894x over previous
"""Trainium2 Bass kernel for a dense transformer encoder block.

Shards across 8 NeuronCores with no collectives: core c handles batch
b=c//2 and query-half qh=c%2 (1024 query rows). K/V are recomputed per
core over the full 2048-row sequence of its batch.

Structure (v2):
- Host pre-transposes x (xt input), folds bv@Wo+bo into the residual
  input xq, g1 into W1, be1 into b1/b2. All exact algebra.
- Phase A: QKV projections split into quanta interleaved between
  attention chunks so the PE never lumps projection work while the
  scalar engine (exp pacemaker) starves.
- Phase B: Wo + residual + LN1 with square/normalize on the scalar
  engine, h kept bf16, bf16 PE transposes for the FFN layout.
- Phase C: FFN1 (u fully resident), FFN2 tq-outer with 32-matmul PSUM
  chains; LN2 + store pipelined under the FFN2 matmuls. W2 prefetched
  on the scalar engine's DMA queue into buffers freed by earlier
  phases.

Numerics: bf16 storage/matmul operands with fp32 PSUM accumulation and
fp32 softmax/layernorm statistics.

Self-contained: needs numpy + the concourse tree at /opt/trn_rl_repo.
"""

import sys

if "/opt/trn_rl_repo" not in sys.path:
    sys.path.insert(0, "/opt/trn_rl_repo")

import numpy as np

B, S, D, H, DK, FFN = 4, 2048, 1024, 16, 64, 4096
P = 128            # partitions
NSQ = S // 2       # local query rows per core (1024)
HP = H // 2        # head pairs (8)
DC = D // P        # d_model chunks (8)
SC = S // P        # sequence chunks (16)
FC = FFN // P      # ffn chunks (32)
EPS = 1e-5

# gbe vector order in the stacked [4, 1, D] input
VG1, VB2, VG2, VBE2 = range(4)

_compiled = None


def _build():
    import os
    PH = int(os.environ.get("KERNEL_PHASES", "9"))
    import concourse.bacc as bacc
    import concourse.tile as tile
    import concourse.mybir as mybir
    from concourse.masks import make_identity

    f32 = mybir.dt.float32
    f32r = mybir.dt.float32r
    bf16 = mybir.dt.bfloat16
    ACT = mybir.ActivationFunctionType
    ALU = mybir.AluOpType

    nc = bacc.Bacc("TRN2", target_bir_lowering=False, debug=False,
                   enable_asserts=False, num_devices=8)

    xt_d = nc.dram_tensor("xt", [P, 4, DC, 512], bf16, kind="ExternalInput")
    xq_d = nc.dram_tensor("xq", [P, DC, D], bf16, kind="ExternalInput")
    wq_d = nc.dram_tensor("wq", [HP, P, DC, P], bf16, kind="ExternalInput")
    wk_d = nc.dram_tensor("wk", [HP, P, DC, P], bf16, kind="ExternalInput")
    wv_d = nc.dram_tensor("wv", [4, P, DC, 256], bf16, kind="ExternalInput")
    bq_d = nc.dram_tensor("bq", [P, HP], f32, kind="ExternalInput")
    bk_d = nc.dram_tensor("bk", [P, HP], f32, kind="ExternalInput")
    wo_d = nc.dram_tensor("wo", [P, DC, D], bf16, kind="ExternalInput")
    w1_d = nc.dram_tensor("w1", [FC, P, DC, P], bf16, kind="ExternalInput")
    b1_d = nc.dram_tensor("b1", [P, FC], f32, kind="ExternalInput")
    w2_d = nc.dram_tensor("w2", [4, P, DC, D], bf16, kind="ExternalInput")
    gbe_d = nc.dram_tensor("gbe", [4, 1, D], bf16, kind="ExternalInput")
    out_d = nc.dram_tensor("out", [P, DC, D], f32, kind="ExternalOutput")

    UDIM = [P, DC, NSQ]   # 16KB arena unit shape (bf16)

    with tile.TileContext(nc) as tc:
        with (
            tc.tile_pool(name="const", bufs=1) as const,
            tc.tile_pool(name="vbc", bufs=1) as vbc,
            tc.tile_pool(name="arena", bufs=1) as arena,
            tc.tile_pool(name="v4p", bufs=1) as v4p,
            tc.tile_pool(name="w1p", bufs=3) as w1p,
            tc.tile_pool(name="small1", bufs=1) as small1,
            tc.tile_pool(name="work", bufs=2) as work,
            tc.tile_pool(name="ps_acc", bufs=2, space="PSUM") as ps_acc,
            tc.tile_pool(name="ps_sc", bufs=2, space="PSUM") as ps_sc,
            tc.tile_pool(name="ps_zt", bufs=2, space="PSUM") as ps_zt,
        ):
            ident_bf = const.tile([P, P], bf16)
            make_identity(nc, ident_bf[:])
            eps_sb = const.tile([P, 1], f32)
            nc.gpsimd.memset(eps_sb[:], float(EPS))

            bq_sb = const.tile([P, HP], f32)
            bk_sb = const.tile([P, HP], f32)
            b1_sb = const.tile([P, FC], f32)
            nc.sync.dma_start(bq_sb[:], bq_d[:])
            nc.sync.dma_start(bk_sb[:], bk_d[:])
            nc.sync.dma_start(b1_sb[:], b1_d[:])

            def make_bcast(idx, tag):
                # broadcast gbe_d[idx] ([1, D]) to [P, D]
                t = vbc.tile([P, D], bf16, tag=tag)
                for sl in range(2):
                    vs = small1.tile([1, 512], bf16, tag="l1", name="vs")
                    nc.sync.dma_start(vs[:],
                                      gbe_d[idx][:, sl * 512:(sl + 1) * 512])
                    nc.gpsimd.partition_broadcast(
                        t[:, sl * 512:(sl + 1) * 512], vs[:])
                return t

            # ---- arena unit tiles (16KB each); tags chain across phases
            xt_lo = arena.tile(UDIM, bf16, tag="A1")   # x^T cols 0..1023
            xt_hi = arena.tile(UDIM, bf16, tag="A2")   # x^T cols 1024..2047
            wk_sb = arena.tile([P, HP, DC, P], bf16, tag="A3")
            wq_sb = arena.tile([P, HP, DC, P], bf16, tag="A4")
            wv_sb = arena.tile([P, 4, DC, 256], bf16, tag="A5")
            ct = arena.tile([P, HP, NSQ], bf16, tag="A6")  # concat^T
            ktqa = arena.tile(UDIM, bf16, tag="A7")  # kt x2, qt x2, at x2

            # x^T DMA split by 512-col chunks so the first projection
            # quantum unblocks early
            for st in range(2):
                nc.sync.dma_start(xt_lo[:, :, st * 512:(st + 1) * 512],
                                  xt_d[:, st])
            nc.sync.dma_start(wk_sb[:, 0], wk_d[0])
            nc.sync.dma_start(wq_sb[:, 0], wq_d[0])
            for st in range(2, 4):
                nc.sync.dma_start(
                    xt_hi[:, :, (st - 2) * 512:(st - 1) * 512],
                    xt_d[:, st])
            nc.sync.dma_start(wv_sb[:, 0], wv_d[0])
            for hp in range(1, HP):
                nc.sync.dma_start(wk_sb[:, hp], wk_d[hp])
                nc.sync.dma_start(wq_sb[:, hp], wq_d[hp])
            for g in range(1, 4):
                nc.sync.dma_start(wv_sb[:, g], wv_d[g])

            # views into the ktqa unit
            def kt_view(hp):
                j = (hp % 2) * 2
                return ktqa[:, j:j + 2, :].rearrange("p a b -> p (a b)")

            def qt_view(hp):
                return ktqa[:, 4 + hp % 2, :]

            def at_view(c):
                return ktqa[:, 6 + c % 2, :].rearrange(
                    "p (h q) -> p h q", h=2)

            def xts(st):
                # [P, DC, 512] view of x^T for sequence 512-chunk st
                src = xt_lo if st < 2 else xt_hi
                return src[:, :, (st % 2) * 512:(st % 2 + 1) * 512]

            def xtc(sc):
                # [P, DC, 128] view of x^T for sequence 128-chunk sc
                src = xt_lo if sc < 8 else xt_hi
                j = sc % 8
                return src[:, :, j * P:(j + 1) * P]

            v4g = [None, None]

            def v4_tile(g):
                t = v4p.tile([P, SC, 4, DK + 1], bf16, tag=f"V{g % 2}")
                nc.vector.memset(t[:, :, :, DK], 1.0)
                v4g[g % 2] = t
                return t

            # ---- projection quanta
            def k_quantum(hp, st):
                def go():
                    pk = ps_acc.tile([P, 512], f32, tag="acc")
                    for dc in range(DC):
                        nc.tensor.matmul(pk[:],
                                         wk_sb[:, hp, dc, :],
                                         xts(st)[:, dc, :],
                                         start=(dc == 0), stop=(dc == DC - 1))
                    nc.vector.tensor_scalar(
                        out=kt_view(hp)[:, st * 512:(st + 1) * 512],
                        in0=pk[:], scalar1=bk_sb[:, hp:hp + 1], scalar2=None,
                        op0=ALU.add)
                return go

            def q_quantum(hp, st):
                def go():
                    pq = ps_acc.tile([P, 512], f32, tag="acc")
                    for dc in range(DC):
                        nc.tensor.matmul(pq[:],
                                         wq_sb[:, hp, dc, :],
                                         xts(st)[:, dc, :],
                                         start=(dc == 0), stop=(dc == DC - 1))
                    nc.vector.tensor_scalar(
                        out=qt_view(hp)[:, st * 512:(st + 1) * 512],
                        in0=pq[:], scalar1=bq_sb[:, hp:hp + 1], scalar2=None,
                        op0=ALU.add)
                return go

            def v_quantum(g, sc):
                def go():
                    pv = ps_acc.tile([P, 256], f32, tag="acc")
                    for dc in range(DC):
                        nc.tensor.matmul(pv[:], xtc(sc)[:, dc, :],
                                         wv_sb[:, g, dc, :],
                                         start=(dc == 0), stop=(dc == DC - 1))
                    nc.vector.tensor_copy(
                        v4g[g % 2][:, sc, :, 0:DK],
                        pv[:].rearrange("p (h k) -> p h k", h=4))
                return go

            pending = []

            def pump(n=1):
                for _ in range(n):
                    if pending:
                        pending.pop(0)()

            # prologue projections for hp=0 (+ first V chunks of group 0)
            if PH >= 2:
                for st in range(4):
                    k_quantum(0, st)()
                for st in range(2):
                    q_quantum(0, st)()
                v4_tile(0)
                for sc in range(4):
                    v_quantum(0, sc)()
                pending += [v_quantum(0, sc) for sc in range(4, SC)]

            # ---- phase A: attention, software-pipelined one chunk deep:
            # scores(i+1) is emitted before Z(i) so the PE computes the next
            # chunk's scores while the scalar engine runs exp(i).
            def normalize_ct(hp, l4):
                # divide hp's unnormalized Z^T slices by their softmax sums;
                # l rows live at 32-aligned partitions of l4
                nc.vector.reciprocal(l4[:], l4[:])
                for j in range(4):
                    sq_t, h = divmod(j, 2)
                    l1 = small1.tile([1, 512], f32, tag="l1", name="l1")
                    nc.vector.tensor_copy(l1[:], l4[32 * j:32 * j + 1, :])
                    bc_l = small1.tile([P, 512], f32, tag="bc_l", name="bc")
                    nc.gpsimd.partition_broadcast(bc_l[:], l1[:])
                    csl = ct[h * DK:(h + 1) * DK, hp,
                             sq_t * 512:(sq_t + 1) * 512]
                    nc.vector.tensor_tensor(csl, csl,
                                            bc_l[h * DK:(h + 1) * DK, :],
                                            ALU.mult)

            chunks = [(hp, sq_t, c) for hp in range(HP)
                      for sq_t in range(NSQ // 512) for c in range(SC)]
            if PH < 2:
                chunks = []

            def emit_scores(hp, sq_t, c):
                scp = ps_sc.tile([P, 2, 512], f32, tag="sc", name="scp")
                kt_hp = kt_view(hp)
                qt_hp = qt_view(hp)
                for h in range(2):
                    nc.tensor.matmul(
                        scp[:, h, :],
                        kt_hp[h * DK:(h + 1) * DK, c * P:(c + 1) * P],
                        qt_hp[h * DK:(h + 1) * DK,
                              sq_t * 512:(sq_t + 1) * 512],
                        start=True, stop=True)
                return scp

            zts = None
            l4_cur = None
            l4_prev = None
            scp_cur = emit_scores(*chunks[0]) if chunks else None
            for i, (hp, sq_t, c) in enumerate(chunks):
                if c == 0:
                    if sq_t == 0:
                        if hp > 0:
                            normalize_ct(hp - 1, l4_prev)
                        l4_cur = small1.tile(
                            [97, 512], f32, tag="l4", name="l4")
                        nc.vector.memset(l4_cur[:], 1.0)
                        l4_prev = l4_cur
                    if hp < HP - 1 and sq_t == 0:
                        pending.extend(k_quantum(hp + 1, st)
                                       for st in range(4))
                        pending.extend(q_quantum(hp + 1, st)
                                       for st in range(2))
                        if hp % 2 == 1:
                            v4_tile(hp // 2 + 1)
                            pending.extend(v_quantum(hp // 2 + 1, sc)
                                           for sc in range(SC))
                    zt0 = ps_zt.tile([DK + 1, 512], f32, tag="zt")
                    zt1 = ps_zt.tile([DK + 1, 512], f32, tag="zt")
                    zts = (zt0, zt1)
                at = at_view(c)
                nc.scalar.activation(at, scp_cur[:], ACT.Exp, scale=0.125)
                if i + 1 < len(chunks):
                    scp_cur = emit_scores(*chunks[i + 1])
                for h in range(2):
                    nc.tensor.matmul(
                        zts[h][:],
                        v4g[(hp // 2) % 2][:, c, (hp % 2) * 2 + h, :],
                        at[:, h, :],
                        start=(c == 0), stop=(c == SC - 1))
                pump(1)
                if c == SC - 1:
                    for h in range(2):
                        j = sq_t * 2 + h
                        nc.vector.tensor_copy(
                            ct[h * DK:(h + 1) * DK, hp,
                               sq_t * 512:(sq_t + 1) * 512],
                            zts[h][0:DK, :])
                        nc.vector.tensor_copy(
                            l4_cur[32 * j:32 * j + 1, :],
                            zts[h][DK:DK + 1, :])

            if PH >= 2:
                pump(len(pending))
                normalize_ct(HP - 1, l4_prev)

            # late-phase weight/data loads into freed arena units
            wo_sb = arena.tile([P, DC, D], bf16, tag="A3")   # over wk
            xq_sb = arena.tile([P, DC, NSQ], bf16, tag="A4")  # over wq
            h_core = arena.tile([P, DC, D], bf16, tag="A5")   # over wv
            if PH >= 3:
                nc.sync.dma_start(wo_sb[:], wo_d[:])
                nc.sync.dma_start(xq_sb[:], xq_d[:])

            # W2 units: prefetched via the scalar engine's DMA queue into
            # buffers freed as phases retire (A1/A2 after projections,
            # A3/A4 after phase B).
            w2u = [arena.tile([P, DC, D], bf16, tag=t, name=f"w2u{t}")
                   for t in ("A1", "A2", "A3", "A4")]
            if PH >= 5:
                for j in range(2):
                    nc.scalar.dma_start(w2u[j][:], w2_d[j])

            # uts units (u^T, relu(W1^T h^T)): A6 freed after Wo, plus
            # three fresh units
            uts = [arena.tile([P, DC, NSQ], bf16, tag=t, name=f"uts{t}")
                   for t in ("A6", "A8", "A9", "A10")]
            ht = arena.tile(UDIM, bf16, tag="A7")  # over ktqa

            g1b = make_bcast(VG1, "g1b") if PH >= 5 else None
            b2b = make_bcast(VB2, "b2b") if PH >= 5 else None
            g2b = make_bcast(VG2, "g2b") if PH >= 5 else None
            be2b = make_bcast(VBE2, "be2b") if PH >= 5 else None

            def ln_stats(res_parts):
                """res_parts: [(res [P,512] f32, rowsum [P,1]), ...2] ->
                (rs [P,1], nmu [P,1]) via scalar-engine square/sqrt."""
                (r0, s0), (r1, s1) = res_parts
                mu = work.tile([P, 1], f32, tag="mu")
                nc.vector.tensor_tensor(mu[:], s0[:], s1[:], ALU.add)
                nc.vector.tensor_scalar_mul(mu[:], mu[:], 1.0 / D)
                ssq0 = work.tile([P, 1], f32, tag="ssq0")
                ssq1 = work.tile([P, 1], f32, tag="ssq1")
                for rsl, ssq in ((r0, ssq0), (r1, ssq1)):
                    sqz = ps_zt.tile([P, 512], f32, tag="zt")
                    nc.scalar.activation(sqz[:], rsl[:], ACT.Square,
                                         accum_out=ssq[:])
                var = work.tile([P, 1], f32, tag="var")
                nc.vector.tensor_tensor(var[:], ssq0[:], ssq1[:], ALU.add)
                nc.vector.tensor_scalar_mul(var[:], var[:], 1.0 / D)
                musq = work.tile([P, 1], f32, tag="musq")
                nc.vector.tensor_mul(musq[:], mu[:], mu[:])
                nc.vector.tensor_sub(var[:], var[:], musq[:])
                sd = work.tile([P, 1], f32, tag="sd")
                nc.scalar.activation(sd[:], var[:], ACT.Sqrt, bias=eps_sb[:])
                rs = work.tile([P, 1], f32, tag="rs")
                nc.vector.reciprocal(rs[:], sd[:])
                nmu = work.tile([P, 1], f32, tag="nmu")
                nc.vector.tensor_mul(nmu[:], mu[:], rs[:])
                nc.vector.tensor_scalar_mul(nmu[:], nmu[:], -1.0)
                return rs, nmu

            # ---- phase B: Wo + residual + LN1 -> h_core (bf16), h^T
            def transpose_tq(tq):
                for dq in range(2):
                    tp = ps_sc.tile([P, 4, P], bf16, tag="sc")
                    for j in range(4):
                        dc = dq * 4 + j
                        nc.tensor.transpose(
                            tp[:, j, :],
                            h_core[:, tq, dc * P:(dc + 1) * P],
                            ident_bf[:])
                    nc.vector.tensor_copy(
                        ht[:, dq * 4:(dq + 1) * 4, tq * P:(tq + 1) * P],
                        tp[:])

            for tq in range(DC if PH >= 3 else 0):
                res_parts = []
                for sl in range(2):
                    pa = ps_acc.tile([P, 512], f32, tag="acc")
                    for hp in range(HP):
                        nc.tensor.matmul(
                            pa[:], ct[:, hp, tq * P:(tq + 1) * P],
                            wo_sb[:, hp, sl * 512:(sl + 1) * 512],
                            start=(hp == 0), stop=(hp == HP - 1))
                    res = work.tile([P, 512], f32, tag=f"r{sl}")
                    rsum = work.tile([P, 1], f32, tag=f"rsum{sl}")
                    nc.vector.scalar_tensor_tensor(
                        out=res[:], in0=pa[:], scalar=1.0,
                        in1=xq_sb[:, tq, sl * 512:(sl + 1) * 512],
                        op0=ALU.mult, op1=ALU.add, accum_out=rsum[:])
                    res_parts.append((res, rsum))
                rs, nmu = ln_stats(res_parts)
                for sl in range(2):
                    nc.scalar.activation(
                        h_core[:, tq, sl * 512:(sl + 1) * 512],
                        res_parts[sl][0][:], ACT.Identity,
                        bias=nmu[:], scale=rs[:])
                if PH >= 4 and tq > 0:
                    transpose_tq(tq - 1)
            if PH >= 4:
                transpose_tq(DC - 1)

            # ---- phase C: FFN1 (full u resident), then tq-outer FFN2+LN2
            for ft in range(FC if PH >= 4 else 0):
                w1t = w1p.tile([P, DC, P], bf16, tag="w1")
                nc.scalar.dma_start(w1t[:], w1_d[ft])
                for st in range(2):
                    pu = ps_acc.tile([P, 512], f32, tag="acc")
                    for dc in range(DC):
                        nc.tensor.matmul(
                            pu[:], w1t[:, dc, :],
                            ht[:, dc, st * 512:(st + 1) * 512],
                            start=(dc == 0), stop=(dc == DC - 1))
                    nc.vector.tensor_scalar(
                        out=uts[ft // DC][:, ft % DC,
                                          st * 512:(st + 1) * 512],
                        in0=pu[:], scalar1=b1_sb[:, ft:ft + 1],
                        scalar2=0.0, op0=ALU.add, op1=ALU.max)
                if PH >= 5 and ft == 0:
                    for j in range(2, 4):
                        nc.scalar.dma_start(w2u[j][:], w2_d[j])

            for tq in range(DC if PH >= 5 else 0):
                res_parts = []
                for sl in range(2):
                    py = ps_acc.tile([P, 512], f32, tag="acc")
                    for fc in range(FC):
                        nc.tensor.matmul(
                            py[:],
                            uts[fc // DC][:, fc % DC, tq * P:(tq + 1) * P],
                            w2u[fc // DC][:, fc % DC,
                                          sl * 512:(sl + 1) * 512],
                            start=(fc == 0), stop=(fc == FC - 1))
                    hs = h_core[:, tq, sl * 512:(sl + 1) * 512]
                    t0 = work.tile([P, 512], f32, tag="t0", bufs=1)
                    nc.vector.tensor_tensor(
                        t0[:], hs, g1b[:, sl * 512:(sl + 1) * 512], ALU.mult)
                    nc.vector.tensor_tensor(
                        t0[:], t0[:], b2b[:, sl * 512:(sl + 1) * 512],
                        ALU.add)
                    z = work.tile([P, 512], f32, tag=f"r{sl}")
                    rsum = work.tile([P, 1], f32, tag=f"rsum{sl}")
                    nc.vector.scalar_tensor_tensor(
                        out=z[:], in0=py[:], scalar=1.0, in1=t0[:],
                        op0=ALU.mult, op1=ALU.add, accum_out=rsum[:])
                    res_parts.append((z, rsum))
                rs, nmu = ln_stats(res_parts)
                for sl in range(2):
                    z = res_parts[sl][0]
                    nc.scalar.activation(z[:], z[:], ACT.Identity,
                                         bias=nmu[:], scale=rs[:])
                    nc.vector.tensor_tensor(
                        z[:], z[:], g2b[:, sl * 512:(sl + 1) * 512],
                        ALU.mult)
                    nc.vector.tensor_tensor(
                        z[:], z[:], be2b[:, sl * 512:(sl + 1) * 512],
                        ALU.add)
                    nc.sync.dma_start(
                        out_d[:, tq, sl * 512:(sl + 1) * 512], z[:])

    nc.compile()
    return nc


def _get_compiled():
    global _compiled
    if _compiled is None:
        _compiled = _build()
    return _compiled


def _host_inputs(inputs):
    """Shared (per-core-identical) weight arrays in kernel layout.

    All big tensors are pre-arranged into the exact SBUF layouts so every
    device DMA is a flat per-partition contiguous copy."""
    import ml_dtypes
    f = np.float32
    bf = ml_dtypes.bfloat16
    cat = lambda w: np.ascontiguousarray(
        np.transpose(np.asarray(w, f), (1, 0, 2)).reshape(D, D))
    vec = lambda k: np.asarray(inputs[k], f).reshape(D)
    Wo = np.asarray(inputs["Wo"], f)
    W1 = np.asarray(inputs["W1"], f)
    # folds: bv@Wo+bo -> residual input (see make_in_maps), g1 -> W1,
    # be1 -> b1/b2
    g1 = vec("g1")
    be1 = vec("be1")
    W1f = W1 * g1[:, None]
    b1f = np.asarray(inputs["b1"], f).reshape(FFN) + be1 @ W1
    b2f = vec("b2") + be1
    gbe = np.stack([vec("g1").reshape(1, D), b2f.reshape(1, D),
                    vec("g2").reshape(1, D), vec("be2").reshape(1, D)],
                   axis=0).astype(bf)
    wq_cat = cat(inputs["Wq"])
    wk_cat = cat(inputs["Wk"])
    wv_cat = cat(inputs["Wv"])
    # [HP, P, DC, P]: w[hp, p, n, m] = cat[n*128+p, hp*128+m]
    whp = lambda w: np.ascontiguousarray(
        w.reshape(DC, P, HP, P).transpose(2, 1, 0, 3).astype(bf))
    # [4, P, DC, 256]
    wvg = np.ascontiguousarray(
        wv_cat.reshape(DC, P, 4, 256).transpose(2, 1, 0, 3).astype(bf))
    return {
        "wq": whp(wq_cat),
        "wk": whp(wk_cat),
        "wv": wvg,
        "bq": np.ascontiguousarray(
            np.asarray(inputs["bq"], f).reshape(HP, P).T),
        "bk": np.ascontiguousarray(
            np.asarray(inputs["bk"], f).reshape(HP, P).T),
        "wo": np.ascontiguousarray(
            Wo.reshape(DC, P, D).transpose(1, 0, 2).astype(bf)),
        "w1": np.ascontiguousarray(
            W1f.reshape(DC, P, FC, P).transpose(2, 1, 0, 3).astype(bf)),
        "b1": np.ascontiguousarray(np.asarray(b1f, f).reshape(FC, P).T),
        "w2": np.ascontiguousarray(
            np.asarray(inputs["W2"], f).reshape(4, DC, P, D)
            .transpose(0, 2, 1, 3).astype(bf)),
        "gbe": np.ascontiguousarray(gbe),
    }


def make_in_maps(inputs):
    import ml_dtypes
    bf = ml_dtypes.bfloat16
    shared = _host_inputs(inputs)
    x = np.asarray(inputs["x"], np.float32)
    bo_eff = (np.asarray(inputs["bo"], np.float32)
              + np.asarray(inputs["bv"], np.float32).reshape(D)
              @ np.asarray(inputs["Wo"], np.float32))
    in_maps = []
    for c in range(8):
        b, qh = c // 2, c % 2
        if qh == 0:
            xb = x[b]
        else:
            xb = np.concatenate([x[b, NSQ:], x[b, :NSQ]], axis=0)
        xT = xb.T  # [D, S]
        xq = xb[:NSQ] + bo_eff  # [NSQ, D]
        in_maps.append({
            "xt": np.ascontiguousarray(
                xT.reshape(DC, P, 4, 512).transpose(1, 2, 0, 3).astype(bf)),
            "xq": np.ascontiguousarray(
                xq.reshape(DC, P, D).transpose(1, 0, 2).astype(bf)),
            **shared,
        })
    return in_maps


def assemble(results):
    out = np.empty((B, S, D), np.float32)
    for c in range(8):
        b, qh = c // 2, c % 2
        o = np.asarray(results[c]["out"])  # [P, DC, D]
        out[b, qh * NSQ:(qh + 1) * NSQ, :] = (
            o.transpose(1, 0, 2).reshape(NSQ, D))
    return out


def run_on_hw(inputs, trace=False, tmpdir=None):
    from concourse.bass_utils import run_bass_kernel_spmd
    nc = _get_compiled()
    res = run_bass_kernel_spmd(nc, make_in_maps(inputs), list(range(8)),
                               trace=trace, tmpdir=tmpdir)
    return assemble(res.results), res


def kernel(**inputs):
    out, _ = run_on_hw(inputs)
    return out


# revision 28
# speedup vs baseline: 1.1005x; 1.0102x over previous
"""Trainium2 Bass kernel for a dense transformer encoder block.

Shards across 8 NeuronCores with no collectives: core c handles batch
b=c//2 and query-half qh=c%2 (1024 query rows). K/V are recomputed per
core over the full 2048-row sequence of its batch.

Structure (v2):
- Host pre-transposes x (xt input), folds bv@Wo+bo into the residual
  input xq, g1 into W1, be1 into b1/b2. All exact algebra.
- Phase A: QKV projections split into quanta interleaved between
  attention chunks so the PE never lumps projection work while the
  scalar engine (exp pacemaker) starves.
- Phase B: Wo + residual + LN1 with square/normalize on the scalar
  engine, h kept bf16, bf16 PE transposes for the FFN layout.
- Phase C: FFN1 (u fully resident), FFN2 tq-outer with 32-matmul PSUM
  chains; LN2 + store pipelined under the FFN2 matmuls. W2 prefetched
  on the scalar engine's DMA queue into buffers freed by earlier
  phases.

Numerics: bf16 storage/matmul operands with fp32 PSUM accumulation and
fp32 softmax/layernorm statistics.

Self-contained: needs numpy + the concourse tree at /opt/trn_rl_repo.
"""

import sys

if "/opt/trn_rl_repo" not in sys.path:
    sys.path.insert(0, "/opt/trn_rl_repo")

import numpy as np

B, S, D, H, DK, FFN = 4, 2048, 1024, 16, 64, 4096
P = 128            # partitions
NSQ = S // 2       # local query rows per core (1024)
HP = H // 2        # head pairs (8)
DC = D // P        # d_model chunks (8)
SC = S // P        # sequence chunks (16)
FC = FFN // P      # ffn chunks (32)
EPS = 1e-5

# gbe vector order in the stacked [4, 1, D] input
VG1, VB2, VG2, VBE2 = range(4)

_compiled = None


def _build():
    import os
    PH = int(os.environ.get("KERNEL_PHASES", "9"))
    import concourse.bacc as bacc
    import concourse.tile as tile
    import concourse.mybir as mybir
    from concourse.masks import make_identity

    f32 = mybir.dt.float32
    f32r = mybir.dt.float32r
    bf16 = mybir.dt.bfloat16
    ACT = mybir.ActivationFunctionType
    ALU = mybir.AluOpType

    nc = bacc.Bacc("TRN2", target_bir_lowering=False, debug=False,
                   enable_asserts=False, num_devices=8)

    xt_d = nc.dram_tensor("xt", [P, 4, DC, 512], bf16, kind="ExternalInput")
    xq_d = nc.dram_tensor("xq", [P, DC, D], bf16, kind="ExternalInput")
    wq_d = nc.dram_tensor("wq", [HP, P, DC, P], bf16, kind="ExternalInput")
    wk_d = nc.dram_tensor("wk", [HP, P, DC, P], bf16, kind="ExternalInput")
    wv_d = nc.dram_tensor("wv", [4, P, DC, 256], bf16, kind="ExternalInput")
    bq_d = nc.dram_tensor("bq", [P, HP], f32, kind="ExternalInput")
    bk_d = nc.dram_tensor("bk", [P, HP], f32, kind="ExternalInput")
    wo_d = nc.dram_tensor("wo", [P, DC, D], bf16, kind="ExternalInput")
    w1_d = nc.dram_tensor("w1", [FC, P, DC, P], bf16, kind="ExternalInput")
    b1_d = nc.dram_tensor("b1", [P, FC], f32, kind="ExternalInput")
    w2_d = nc.dram_tensor("w2", [4, P, DC, D], bf16, kind="ExternalInput")
    gbe_d = nc.dram_tensor("gbe", [4, 1, D], bf16, kind="ExternalInput")
    out_d = nc.dram_tensor("out", [P, DC, D], f32, kind="ExternalOutput")

    UDIM = [P, DC, NSQ]   # 16KB arena unit shape (bf16)

    with tile.TileContext(nc) as tc:
        with (
            tc.tile_pool(name="const", bufs=1) as const,
            tc.tile_pool(name="vbc", bufs=1) as vbc,
            tc.tile_pool(name="arena", bufs=1) as arena,
            tc.tile_pool(name="v4p", bufs=1) as v4p,
            tc.tile_pool(name="w1p", bufs=3) as w1p,
            tc.tile_pool(name="small1", bufs=1) as small1,
            tc.tile_pool(name="work", bufs=2) as work,
            tc.tile_pool(name="ps_acc", bufs=2, space="PSUM") as ps_acc,
            tc.tile_pool(name="ps_sc", bufs=2, space="PSUM") as ps_sc,
            tc.tile_pool(name="ps_zt", bufs=2, space="PSUM") as ps_zt,
        ):
            ident_bf = const.tile([P, P], bf16)
            make_identity(nc, ident_bf[:])
            eps_sb = const.tile([P, 1], f32)
            nc.gpsimd.memset(eps_sb[:], float(EPS))

            bq_sb = const.tile([P, HP], f32)
            bk_sb = const.tile([P, HP], f32)
            b1_sb = const.tile([P, FC], f32)
            nc.sync.dma_start(bq_sb[:], bq_d[:])
            nc.sync.dma_start(bk_sb[:], bk_d[:])
            nc.sync.dma_start(b1_sb[:], b1_d[:])

            def make_bcast(idx, tag):
                # broadcast gbe_d[idx] ([1, D]) to [P, D]
                t = vbc.tile([P, D], bf16, tag=tag)
                for sl in range(2):
                    vs = small1.tile([1, 512], bf16, tag="l1", name="vs")
                    nc.sync.dma_start(vs[:],
                                      gbe_d[idx][:, sl * 512:(sl + 1) * 512])
                    nc.gpsimd.partition_broadcast(
                        t[:, sl * 512:(sl + 1) * 512], vs[:])
                return t

            # ---- arena unit tiles (16KB each); tags chain across phases
            xt_lo = arena.tile([P, 2, DC, 512], bf16, tag="A1")  # s 0..1023
            xt_hi = arena.tile([P, 2, DC, 512], bf16, tag="A2")  # s 1024..2047
            wk_sb = arena.tile([P, HP, DC, P], bf16, tag="A3")
            wq_sb = arena.tile([P, HP, DC, P], bf16, tag="A4")
            wv_sb = arena.tile([P, 4, DC, 256], bf16, tag="A5")
            ct = arena.tile([P, HP, NSQ], bf16, tag="A6")  # concat^T
            ktqa = arena.tile(UDIM, bf16, tag="A7")  # kt x2, qt x2, at x2

            # x^T DMA split by 512-col chunks so the first projection
            # quantum unblocks early
            for st in range(2):
                nc.sync.dma_start(xt_lo[:, st], xt_d[:, st])
            nc.sync.dma_start(wk_sb[:, 0], wk_d[0])
            nc.sync.dma_start(wq_sb[:, 0], wq_d[0])
            for st in range(2, 4):
                nc.sync.dma_start(xt_hi[:, st - 2], xt_d[:, st])
            nc.sync.dma_start(wv_sb[:, 0], wv_d[0])
            for hp in range(1, HP):
                nc.sync.dma_start(wk_sb[:, hp], wk_d[hp])
                nc.sync.dma_start(wq_sb[:, hp], wq_d[hp])
            for g in range(1, 4):
                nc.sync.dma_start(wv_sb[:, g], wv_d[g])

            # views into the ktqa unit
            def kt_view(hp):
                j = (hp % 2) * 2
                return ktqa[:, j:j + 2, :].rearrange("p a b -> p (a b)")

            def qt_view(hp):
                return ktqa[:, 4 + hp % 2, :]

            def at_view(c):
                return ktqa[:, 6 + c % 2, :].rearrange(
                    "p (h q) -> p h q", h=2)

            def xts(st):
                # [P, DC, 512] view of x^T for sequence 512-chunk st
                src = xt_lo if st < 2 else xt_hi
                return src[:, st % 2]

            def xtc(sc):
                # [P, DC, 128] view of x^T for sequence 128-chunk sc
                st = sc // 4
                src = xt_lo if st < 2 else xt_hi
                j = sc % 4
                return src[:, st % 2, :, j * P:(j + 1) * P]

            v4g = [None, None]

            def v4_tile(g):
                t = v4p.tile([P, SC, 4, DK + 1], bf16, tag=f"V{g % 2}")
                nc.vector.memset(t[:, :, :, DK], 1.0)
                v4g[g % 2] = t
                return t

            # ---- projection quanta
            def k_quantum(hp, st):
                def go():
                    pk = ps_acc.tile([P, 512], f32, tag="acc")
                    for dc in range(DC):
                        nc.tensor.matmul(pk[:],
                                         wk_sb[:, hp, dc, :],
                                         xts(st)[:, dc, :],
                                         start=(dc == 0), stop=(dc == DC - 1))
                    nc.vector.tensor_scalar(
                        out=kt_view(hp)[:, st * 512:(st + 1) * 512],
                        in0=pk[:], scalar1=bk_sb[:, hp:hp + 1], scalar2=None,
                        op0=ALU.add)
                return go

            def q_quantum(hp, st):
                def go():
                    pq = ps_acc.tile([P, 512], f32, tag="acc")
                    for dc in range(DC):
                        nc.tensor.matmul(pq[:],
                                         wq_sb[:, hp, dc, :],
                                         xts(st)[:, dc, :],
                                         start=(dc == 0), stop=(dc == DC - 1))
                    nc.vector.tensor_scalar(
                        out=qt_view(hp)[:, st * 512:(st + 1) * 512],
                        in0=pq[:], scalar1=bq_sb[:, hp:hp + 1], scalar2=None,
                        op0=ALU.add)
                return go

            def v_quantum(g, sc):
                def go():
                    pv = ps_acc.tile([P, 256], f32, tag="acc")
                    for dc in range(DC):
                        nc.tensor.matmul(pv[:], xtc(sc)[:, dc, :],
                                         wv_sb[:, g, dc, :],
                                         start=(dc == 0), stop=(dc == DC - 1))
                    nc.vector.tensor_copy(
                        v4g[g % 2][:, sc, :, 0:DK],
                        pv[:].rearrange("p (h k) -> p h k", h=4))
                return go

            pending = []

            def pump(n=1):
                for _ in range(n):
                    if pending:
                        pending.pop(0)()

            # prologue projections for hp=0 (+ first V chunks of group 0)
            if PH >= 2:
                for st in range(4):
                    k_quantum(0, st)()
                for st in range(2):
                    q_quantum(0, st)()
                v4_tile(0)
                for sc in range(4):
                    v_quantum(0, sc)()
                pending += [v_quantum(0, sc) for sc in range(4, SC)]

            # ---- phase A: attention, software-pipelined one chunk deep:
            # scores(i+1) is emitted before Z(i) so the PE computes the next
            # chunk's scores while the scalar engine runs exp(i).
            def norm_piece(hp, j, l4):
                # ct slice /= softmax sum; l4 row 32j holds the reciprocal
                sq_t, h = divmod(j, 2)
                l1 = small1.tile([1, 512], f32, tag="l1", name="l1")
                nc.vector.tensor_copy(l1[:], l4[32 * j:32 * j + 1, :])
                bc_l = small1.tile([P, 512], f32, tag="bc_l", name="bc")
                nc.gpsimd.partition_broadcast(bc_l[:], l1[:])
                csl = ct[h * DK:(h + 1) * DK, hp,
                         sq_t * 512:(sq_t + 1) * 512]
                nc.vector.tensor_tensor(csl, csl,
                                        bc_l[h * DK:(h + 1) * DK, :],
                                        ALU.mult)

            def normalize_ct(hp, l4):
                nc.vector.reciprocal(l4[:], l4[:])
                for j in range(4):
                    norm_piece(hp, j, l4)

            chunks = [(hp, sq_t, c) for hp in range(HP)
                      for sq_t in range(NSQ // 512) for c in range(SC)]
            if PH < 2:
                chunks = []

            def emit_scores(hp, sq_t, c):
                scp = ps_sc.tile([P, 2, 512], f32, tag="sc", name="scp")
                kt_hp = kt_view(hp)
                qt_hp = qt_view(hp)
                for h in range(2):
                    nc.tensor.matmul(
                        scp[:, h, :],
                        kt_hp[h * DK:(h + 1) * DK, c * P:(c + 1) * P],
                        qt_hp[h * DK:(h + 1) * DK,
                              sq_t * 512:(sq_t + 1) * 512],
                        start=True, stop=True)
                return scp

            zts = None
            l4_cur = None
            l4_prev = None
            scp_cur = emit_scores(*chunks[0]) if chunks else None
            for i, (hp, sq_t, c) in enumerate(chunks):
                if c == 0:
                    if sq_t == 0:
                        if hp > 0:
                            normalize_ct(hp - 1, l4_prev)
                        l4_cur = small1.tile(
                            [97, 512], f32, tag="l4", name="l4")
                        nc.vector.memset(l4_cur[:], 1.0)
                        l4_prev = l4_cur
                    if hp < HP - 1 and sq_t == 0:
                        pending.extend(k_quantum(hp + 1, st)
                                       for st in range(4))
                        pending.extend(q_quantum(hp + 1, st)
                                       for st in range(2))
                        if hp % 2 == 1:
                            v4_tile(hp // 2 + 1)
                            pending.extend(v_quantum(hp // 2 + 1, sc)
                                           for sc in range(SC))
                    zt0 = ps_zt.tile([DK + 1, 512], f32, tag="zt")
                    zt1 = ps_zt.tile([DK + 1, 512], f32, tag="zt")
                    zts = (zt0, zt1)
                at = at_view(c)
                nc.scalar.activation(at, scp_cur[:], ACT.Exp, scale=0.125)
                if i + 1 < len(chunks):
                    scp_cur = emit_scores(*chunks[i + 1])
                for h in range(2):
                    nc.tensor.matmul(
                        zts[h][:],
                        v4g[(hp // 2) % 2][:, c, (hp % 2) * 2 + h, :],
                        at[:, h, :],
                        start=(c == 0), stop=(c == SC - 1))
                pump(1)
                if c == SC - 1:
                    for h in range(2):
                        j = sq_t * 2 + h
                        nc.vector.tensor_copy(
                            ct[h * DK:(h + 1) * DK, hp,
                               sq_t * 512:(sq_t + 1) * 512],
                            zts[h][0:DK, :])
                        nc.vector.tensor_copy(
                            l4_cur[32 * j:32 * j + 1, :],
                            zts[h][DK:DK + 1, :])
                    if hp == HP - 1:
                        # last head-pair: normalize per sq_t so the chain
                        # hides under the remaining attention chunks
                        nc.vector.reciprocal(
                            l4_cur[64 * sq_t:64 * sq_t + 33, :],
                            l4_cur[64 * sq_t:64 * sq_t + 33, :])
                        norm_piece(hp, sq_t * 2, l4_cur)
                        norm_piece(hp, sq_t * 2 + 1, l4_cur)

            if PH >= 2:
                pump(len(pending))

            # late-phase weight/data loads into freed arena units
            wo_sb = arena.tile([P, DC, D], bf16, tag="A3")   # over wk
            xq_sb = arena.tile([P, DC, NSQ], bf16, tag="A4")  # over wq
            h_core = arena.tile([P, DC, D], bf16, tag="A5")   # over wv
            if PH >= 3:
                nc.sync.dma_start(wo_sb[:], wo_d[:])
                nc.sync.dma_start(xq_sb[:], xq_d[:])

            # W2 units: prefetched via the scalar engine's DMA queue into
            # buffers freed as phases retire (A1/A2 after projections,
            # A3/A4 after phase B).
            w2u = [arena.tile([P, DC, D], bf16, tag=t, name=f"w2u{t}")
                   for t in ("A1", "A2", "A3", "A4")]
            if PH >= 5:
                for j in range(2):
                    nc.scalar.dma_start(w2u[j][:], w2_d[j])

            # uts units (u^T, relu(W1^T h^T)): A6 freed after Wo, plus
            # three fresh units
            uts = [arena.tile([P, DC, NSQ], bf16, tag=t, name=f"uts{t}")
                   for t in ("A6", "A8", "A9", "A10")]
            ht = arena.tile(UDIM, bf16, tag="A7")  # over ktqa

            g1b = make_bcast(VG1, "g1b") if PH >= 5 else None
            b2b = make_bcast(VB2, "b2b") if PH >= 5 else None
            g2b = make_bcast(VG2, "g2b") if PH >= 5 else None
            be2b = make_bcast(VBE2, "be2b") if PH >= 5 else None

            def ln_stats(res_parts):
                """res_parts: [(res [P,512] f32, rowsum [P,1]), ...2] ->
                (rs [P,1], nmu [P,1]) via scalar-engine square/sqrt."""
                (r0, s0), (r1, s1) = res_parts
                mu = work.tile([P, 1], f32, tag="mu")
                nc.vector.tensor_tensor(mu[:], s0[:], s1[:], ALU.add)
                nc.vector.tensor_scalar_mul(mu[:], mu[:], 1.0 / D)
                ssq0 = work.tile([P, 1], f32, tag="ssq0")
                ssq1 = work.tile([P, 1], f32, tag="ssq1")
                for rsl, ssq in ((r0, ssq0), (r1, ssq1)):
                    sqz = ps_zt.tile([P, 512], f32, tag="zt")
                    nc.scalar.activation(sqz[:], rsl[:], ACT.Square,
                                         accum_out=ssq[:])
                var = work.tile([P, 1], f32, tag="var")
                nc.vector.tensor_tensor(var[:], ssq0[:], ssq1[:], ALU.add)
                nc.vector.tensor_scalar_mul(var[:], var[:], 1.0 / D)
                musq = work.tile([P, 1], f32, tag="musq")
                nc.vector.tensor_mul(musq[:], mu[:], mu[:])
                nc.vector.tensor_sub(var[:], var[:], musq[:])
                sd = work.tile([P, 1], f32, tag="sd")
                nc.scalar.activation(sd[:], var[:], ACT.Sqrt, bias=eps_sb[:])
                rs = work.tile([P, 1], f32, tag="rs")
                nc.vector.reciprocal(rs[:], sd[:])
                nmu = work.tile([P, 1], f32, tag="nmu")
                nc.vector.tensor_mul(nmu[:], mu[:], rs[:])
                nc.vector.tensor_scalar_mul(nmu[:], nmu[:], -1.0)
                return rs, nmu

            # ---- phase B: Wo + residual + LN1 -> h_core (bf16), h^T
            def transpose_tq(tq):
                for dq in range(2):
                    tp = ps_sc.tile([P, 4, P], bf16, tag="sc")
                    for j in range(4):
                        dc = dq * 4 + j
                        nc.tensor.transpose(
                            tp[:, j, :],
                            h_core[:, tq, dc * P:(dc + 1) * P],
                            ident_bf[:])
                    nc.vector.tensor_copy(
                        ht[:, dq * 4:(dq + 1) * 4, tq * P:(tq + 1) * P],
                        tp[:])

            for tq in range(DC if PH >= 3 else 0):
                res_parts = []
                for sl in range(2):
                    pa = ps_acc.tile([P, 512], f32, tag="acc")
                    for hp in range(HP):
                        nc.tensor.matmul(
                            pa[:], ct[:, hp, tq * P:(tq + 1) * P],
                            wo_sb[:, hp, sl * 512:(sl + 1) * 512],
                            start=(hp == 0), stop=(hp == HP - 1))
                    res = work.tile([P, 512], f32, tag=f"r{sl}")
                    rsum = work.tile([P, 1], f32, tag=f"rsum{sl}")
                    nc.vector.scalar_tensor_tensor(
                        out=res[:], in0=pa[:], scalar=1.0,
                        in1=xq_sb[:, tq, sl * 512:(sl + 1) * 512],
                        op0=ALU.mult, op1=ALU.add, accum_out=rsum[:])
                    res_parts.append((res, rsum))
                rs, nmu = ln_stats(res_parts)
                for sl in range(2):
                    nc.scalar.activation(
                        h_core[:, tq, sl * 512:(sl + 1) * 512],
                        res_parts[sl][0][:], ACT.Identity,
                        bias=nmu[:], scale=rs[:])
                if PH >= 4 and tq > 0:
                    transpose_tq(tq - 1)
            if PH >= 4:
                transpose_tq(DC - 1)

            # ---- phase C: FFN1 (full u resident), then tq-outer FFN2+LN2
            for ft in range(FC if PH >= 4 else 0):
                w1t = w1p.tile([P, DC, P], bf16, tag="w1")
                nc.scalar.dma_start(w1t[:], w1_d[ft])
                for st in range(2):
                    pu = ps_acc.tile([P, 512], f32, tag="acc")
                    for dc in range(DC):
                        nc.tensor.matmul(
                            pu[:], w1t[:, dc, :],
                            ht[:, dc, st * 512:(st + 1) * 512],
                            start=(dc == 0), stop=(dc == DC - 1))
                    nc.vector.tensor_scalar(
                        out=uts[ft // DC][:, ft % DC,
                                          st * 512:(st + 1) * 512],
                        in0=pu[:], scalar1=b1_sb[:, ft:ft + 1],
                        scalar2=0.0, op0=ALU.add, op1=ALU.max)
                if PH >= 5 and ft == 0:
                    for j in range(2, 4):
                        nc.scalar.dma_start(w2u[j][:], w2_d[j])

            for tq in range(DC if PH >= 5 else 0):
                res_parts = []
                for sl in range(2):
                    py = ps_acc.tile([P, 512], f32, tag="acc")
                    for fc in range(FC):
                        nc.tensor.matmul(
                            py[:],
                            uts[fc // DC][:, fc % DC, tq * P:(tq + 1) * P],
                            w2u[fc // DC][:, fc % DC,
                                          sl * 512:(sl + 1) * 512],
                            start=(fc == 0), stop=(fc == FC - 1))
                    hs = h_core[:, tq, sl * 512:(sl + 1) * 512]
                    t0 = work.tile([P, 512], f32, tag="t0", bufs=1)
                    nc.vector.tensor_tensor(
                        t0[:], hs, g1b[:, sl * 512:(sl + 1) * 512], ALU.mult)
                    nc.vector.tensor_tensor(
                        t0[:], t0[:], b2b[:, sl * 512:(sl + 1) * 512],
                        ALU.add)
                    z = work.tile([P, 512], f32, tag=f"r{sl}")
                    rsum = work.tile([P, 1], f32, tag=f"rsum{sl}")
                    nc.vector.scalar_tensor_tensor(
                        out=z[:], in0=py[:], scalar=1.0, in1=t0[:],
                        op0=ALU.mult, op1=ALU.add, accum_out=rsum[:])
                    res_parts.append((z, rsum))
                rs, nmu = ln_stats(res_parts)
                for sl in range(2):
                    z = res_parts[sl][0]
                    nc.scalar.activation(z[:], z[:], ACT.Identity,
                                         bias=nmu[:], scale=rs[:])
                    nc.vector.tensor_tensor(
                        z[:], z[:], g2b[:, sl * 512:(sl + 1) * 512],
                        ALU.mult)
                    nc.vector.tensor_tensor(
                        z[:], z[:], be2b[:, sl * 512:(sl + 1) * 512],
                        ALU.add)
                    nc.sync.dma_start(
                        out_d[:, tq, sl * 512:(sl + 1) * 512], z[:])

    nc.compile()
    return nc


def _get_compiled():
    global _compiled
    if _compiled is None:
        _compiled = _build()
    return _compiled


def _host_inputs(inputs):
    """Shared (per-core-identical) weight arrays in kernel layout.

    All big tensors are pre-arranged into the exact SBUF layouts so every
    device DMA is a flat per-partition contiguous copy."""
    import ml_dtypes
    f = np.float32
    bf = ml_dtypes.bfloat16
    cat = lambda w: np.ascontiguousarray(
        np.transpose(np.asarray(w, f), (1, 0, 2)).reshape(D, D))
    vec = lambda k: np.asarray(inputs[k], f).reshape(D)
    Wo = np.asarray(inputs["Wo"], f)
    W1 = np.asarray(inputs["W1"], f)
    # folds: bv@Wo+bo -> residual input (see make_in_maps), g1 -> W1,
    # be1 -> b1/b2
    g1 = vec("g1")
    be1 = vec("be1")
    W1f = W1 * g1[:, None]
    b1f = np.asarray(inputs["b1"], f).reshape(FFN) + be1 @ W1
    b2f = vec("b2") + be1
    gbe = np.stack([vec("g1").reshape(1, D), b2f.reshape(1, D),
                    vec("g2").reshape(1, D), vec("be2").reshape(1, D)],
                   axis=0).astype(bf)
    wq_cat = cat(inputs["Wq"])
    wk_cat = cat(inputs["Wk"])
    wv_cat = cat(inputs["Wv"])
    # [HP, P, DC, P]: w[hp, p, n, m] = cat[n*128+p, hp*128+m]
    whp = lambda w: np.ascontiguousarray(
        w.reshape(DC, P, HP, P).transpose(2, 1, 0, 3).astype(bf))
    # [4, P, DC, 256]
    wvg = np.ascontiguousarray(
        wv_cat.reshape(DC, P, 4, 256).transpose(2, 1, 0, 3).astype(bf))
    return {
        "wq": whp(wq_cat),
        "wk": whp(wk_cat),
        "wv": wvg,
        "bq": np.ascontiguousarray(
            np.asarray(inputs["bq"], f).reshape(HP, P).T),
        "bk": np.ascontiguousarray(
            np.asarray(inputs["bk"], f).reshape(HP, P).T),
        "wo": np.ascontiguousarray(
            Wo.reshape(DC, P, D).transpose(1, 0, 2).astype(bf)),
        "w1": np.ascontiguousarray(
            W1f.reshape(DC, P, FC, P).transpose(2, 1, 0, 3).astype(bf)),
        "b1": np.ascontiguousarray(np.asarray(b1f, f).reshape(FC, P).T),
        "w2": np.ascontiguousarray(
            np.asarray(inputs["W2"], f).reshape(4, DC, P, D)
            .transpose(0, 2, 1, 3).astype(bf)),
        "gbe": np.ascontiguousarray(gbe),
    }


def make_in_maps(inputs):
    import ml_dtypes
    bf = ml_dtypes.bfloat16
    shared = _host_inputs(inputs)
    x = np.asarray(inputs["x"], np.float32)
    bo_eff = (np.asarray(inputs["bo"], np.float32)
              + np.asarray(inputs["bv"], np.float32).reshape(D)
              @ np.asarray(inputs["Wo"], np.float32))
    in_maps = []
    for c in range(8):
        b, qh = c // 2, c % 2
        if qh == 0:
            xb = x[b]
        else:
            xb = np.concatenate([x[b, NSQ:], x[b, :NSQ]], axis=0)
        xT = xb.T  # [D, S]
        xq = xb[:NSQ] + bo_eff  # [NSQ, D]
        in_maps.append({
            "xt": np.ascontiguousarray(
                xT.reshape(DC, P, 4, 512).transpose(1, 2, 0, 3).astype(bf)),
            "xq": np.ascontiguousarray(
                xq.reshape(DC, P, D).transpose(1, 0, 2).astype(bf)),
            **shared,
        })
    return in_maps


def assemble(results):
    out = np.empty((B, S, D), np.float32)
    for c in range(8):
        b, qh = c // 2, c % 2
        o = np.asarray(results[c]["out"])  # [P, DC, D]
        out[b, qh * NSQ:(qh + 1) * NSQ, :] = (
            o.transpose(1, 0, 2).reshape(NSQ, D))
    return out


def run_on_hw(inputs, trace=False, tmpdir=None):
    from concourse.bass_utils import run_bass_kernel_spmd
    nc = _get_compiled()
    res = run_bass_kernel_spmd(nc, make_in_maps(inputs), list(range(8)),
                               trace=trace, tmpdir=tmpdir)
    return assemble(res.results), res


def kernel(**inputs):
    out, _ = run_on_hw(inputs)
    return out
